# revision 35
# baseline (speedup 1.0000x reference)
"""Trainium2 Bass kernel for an AttentionBlock (GroupNorm -> 1-head attention -> proj -> residual).

Problem: hidden_states (4, 512, 64, 64) fp32; GroupNorm(32 groups) then
single-head attention over S=4096 tokens with head_dim=C=512, output
projection, residual add.

Sharding: 8 cores = 4 batch elements x 2 query-halves. Each core:
 - receives the full [512, 4096] (channels x spatial) slab for its batch
   element, spatially rotated so that *its* 2048 queries are columns 0:2048
   (attention is permutation-invariant over keys, so every core can run the
   identical SPMD program);
 - computes GroupNorm + K/V for all 4096 tokens (redundant x2, cheap) and
   Q only for its half;
 - computes scores^T (keys-on-partition layout), exp, attn @ V, out-proj,
   residual -- no on-chip transposes anywhere.

Numerics: fp8(e4m3) matmul operands with DoubleRow perf mode (two 128-row
k-tiles contracted per PE pass, ~1.9x TensorE throughput over fp16) and fp32
PSUM accumulation. Weights host-pre-scaled (wq,wk x16; wv x4; wo x16) to
keep fp8 operands out of the subnormal range; the scales cancel through the
softmax-denominator constant (64). Softmax without max-subtraction (scores
~ N(0,1)) with a constant exp-bias of -4; normalization deferred past the
output projection. bv and bk fold away algebraically (bv into bo on the
host; k-bias is softmax-invariant but still applied -- it rides the kt copy
for free). Measured end-to-end max-rel error vs fp32 reference: 6.2e-3.

Schedule highlights:
 - PE warmup paced by the x-DMA stream (junk matmuls consuming each arriving
   quarter) keeps HAM at K=8/8 through the GroupNorm latency;
 - GN stats: per-half Square+accum on ACT pipelined with the DMA, fp16
   pairwise-fold sums on DVE, one [128,8] group-averaging matmul, batched
   scale/shift chain;
 - normalize and the K/V/Q projections run token-quarter-major; projection
   epilogues accumulate jc-pairs in 2-bank PSUM tiles so each PSUM->SBUF
   copy is a single 1024-wide op alternating ACT/DVE (projection phase is
   PE-bound);
 - attention: per 512-query chunk, 16 key-block pairs; scores into
   single-bank PSUM tiles (3-deep rotation), exp on ACT into the fp8
   [128,2,512] pair layout attnV's DoubleRow rhs needs; softmax denominator
   accumulated on PE (fp8 ones-matmul, one pair behind) for pairs 0..7 and
   on DVE (fp16) for pairs 8..15 into a dedicated PSUM bank; depth-6 score
   prologue rides chunk boundaries; per-chunk epilogue (reciprocal, attn-out
   copies, out-proj, residual fuse) deferred into the next chunk's loop.

Measured on 8 axon TRN2 cores: ~237us HW exec (~444us for the fp16
non-DoubleRow predecessor); TensorMatrix ~83% busy, ~186us of which is
within ~3% of the DoubleRow MATMUL roofline for the instruction mix.
"""

from contextlib import ExitStack

import ml_dtypes
import numpy as np

import concourse.bacc as bacc
import concourse.tile as tile
from concourse import mybir
from concourse.bass_utils import run_bass_kernel_spmd

F32 = mybir.dt.float32
F16 = mybir.dt.float16
F8 = mybir.dt.float8e4
F8NP = ml_dtypes.float8_e4m3
DR = mybir.MatmulPerfMode.DoubleRow

B = 4
C = 512
S = 4096  # 64*64 tokens
SH = S // 2  # tokens per core (query half)
GROUPS = 32
GSIZE = C // GROUPS  # 16 channels per group
EPS = 1e-6
CT = C // 128  # 4 channel tiles
SCALE = 1.0 / np.sqrt(np.float32(C))
EXPBIAS = -4.0  # constant max-substitute inside exp; cancels in normalization

QKSCALE = 16.0  # host pre-scale on wq/wk/bq/bk (fp8 range use)
VSCALE = 4.0  # host pre-scale on wv/bv (keeps unnormalized attn@V in fp8 range)
OSCALE = 16.0  # host pre-scale on wo
ONES_VAL = VSCALE * OSCALE  # denominator broadcast constant; cancels v/o scales
EXPSCALE = float(SCALE / (QKSCALE * QKSCALE))  # exp() input scale on raw scores

N_CORES = 8


def _build_kernel(ctx: ExitStack, tc: tile.TileContext, d):
    nc = tc.nc
    mult = mybir.AluOpType.mult
    add = mybir.AluOpType.add
    subtract = mybir.AluOpType.subtract
    Act = mybir.ActivationFunctionType

    cst = ctx.enter_context(tc.tile_pool(name="cst", bufs=1))
    xin = ctx.enter_context(tc.tile_pool(name="xin", bufs=3))
    gnp = ctx.enter_context(tc.tile_pool(name="gnp", bufs=4))
    big = ctx.enter_context(tc.tile_pool(name="big", bufs=1))
    expp = ctx.enter_context(tc.tile_pool(name="expp", bufs=8))
    smal = ctx.enter_context(tc.tile_pool(name="smal", bufs=2))
    resp = ctx.enter_context(tc.tile_pool(name="resp", bufs=2))
    finp = ctx.enter_context(tc.tile_pool(name="finp", bufs=2))

    x_d = d["x"]  # fp16 copy of the input slab: GN stats + matmul path
    # sync DMA queue order: channel tile 0 first (it heads the GroupNorm
    # pipeline), then the tiny GN constants it needs, then the other tiles.
    # Four sub-DMAs per tile so bn_stats starts on the first quarter early;
    # each tile gets its own slot so all transfers issue immediately.
    x_tiles = []
    for t in range(CT):
        x_t = xin.tile([128, S], F16, tag=f"xt{t}", name=f"xt{t}", bufs=1)
        x_tiles.append(x_t)

    def dma_x(t):
        for h in range(4):
            nc.sync.dma_start(
                out=x_tiles[t][:, h * (S // 4) : (h + 1) * (S // 4)],
                in_=x_d[t * 128 : (t + 1) * 128, h * (S // 4) : (h + 1) * (S // 4)],
            )

    dma_x(0)
    gmat_raw = cst.tile([128, 128], F32, tag="gmat_raw")
    nc.sync.dma_start(out=gmat_raw[:], in_=d["gmat"][:])
    gw_sb = cst.tile([128, CT], F32, tag="gw")
    nc.sync.dma_start(out=gw_sb[:], in_=d["gw2"][:])
    gb_sb = cst.tile([128, CT], F32, tag="gb")
    nc.sync.dma_start(out=gb_sb[:], in_=d["gb2"][:])
    for t in range(1, CT):
        dma_x(t)

    # ---- constants / weights to SBUF (gpsimd DMA queue; overlaps x).
    # Order = first-use order: K/Q/V weights gate the projections,
    # biases gate the PSUM->SBUF copies a bit later, wo3/bo much later.
    wq3 = cst.tile([128, CT, C], F8, tag="wq3")
    wk3 = cst.tile([128, CT, C], F8, tag="wk3")
    wv3 = cst.tile([128, CT, C], F8, tag="wv3")
    wo3 = cst.tile([128, CT, C], F8, tag="wo3")
    for w_sb, w_d in ((wk3, d["wkt"]), (wq3, d["wqt"]), (wv3, d["wvt"])):
        nc.gpsimd.dma_start(out=w_sb[:], in_=w_d.rearrange("(t p) o -> p t o", p=128))
    bq_sb = cst.tile([128, CT], F32, tag="bq")
    bk_sb = cst.tile([128, CT], F32, tag="bk")
    bo_sb = cst.tile([128, CT], F32, tag="bo")
    for t_sb, t_d in ((bk_sb, d["bk2"]), (bq_sb, d["bq2"]), (bo_sb, d["bo2"])):
        nc.gpsimd.dma_start(out=t_sb[:], in_=t_d[:])
    nc.gpsimd.dma_start(out=wo3[:], in_=d["wot"].rearrange("(t p) o -> p t o", p=128))
    # staging copy: the first PE matmul then depends only on the DVE
    # semaphore (S3_LW allows a single wait)
    gmat_sb = cst.tile([128, 128], F32, tag="gmat")
    nc.vector.tensor_copy(out=gmat_sb[:], in_=gmat_raw[:])
    ones8 = cst.tile([128, 2, 128], F8, tag="ones8")
    nc.vector.memset(ones8[:], float(ONES_VAL))
    ones16 = cst.tile([128, 128], F16, tag="ones16")
    nc.vector.memset(ones16[:], 1.0)
    ones16d = cst.tile([128, 128], F16, tag="ones16d")
    nc.vector.memset(ones16d[:], float(ONES_VAL))
    eps_t = cst.tile([128, 1], F32, tag="epsc")
    nc.vector.memset(eps_t[:], float(EPS))
    expb_t = cst.tile([128, 1], F32, tag="expb")
    nc.vector.memset(expb_t[:], float(EXPBIAS))

    # proj-phase PSUM pool: 6 banks; scoped so its banks are released to the
    # attention pools afterwards
    proj_ctx = ExitStack()
    pjsum = proj_ctx.enter_context(tc.tile_pool(name="pjsum", bufs=3, space="PSUM"))

    # PE warmup, paced by the x DMA: a short front-fill of ones matmuls,
    # then junk matmuls reading each arriving x quarter -- TensorE tracks the
    # transfer stream (HAM stays K=8/8) instead of idling before the first
    # projection.
    wu = pjsum.tile([128, 512], F32, tag="wu", bufs=1)
    for _ in range(55):
        nc.tensor.matmul(
            wu[:, 0:128], lhsT=ones8[:, 0, :], rhs=ones8[:, 0, :], start=True, stop=True
        )
    for t in range(CT):
        for h in range(4):
            for r in range(3):
                nc.tensor.matmul(
                    wu[:],
                    lhsT=ones16[:],
                    rhs=x_tiles[t][:, h * 1024 + r * 170 : h * 1024 + r * 170 + 512],
                    start=True,
                    stop=True,
                )
    # ---- GroupNorm ----
    # Pass 1 (pipelined with the x DMA): per-quarter sum-of-squares on ACT
    # (accum_out; junk main output lands in the xg3 slot, overwritten by the
    # normalize pass) and fp16 pairwise-fold sums on DVE. All four tiles'
    # raw [sum, sumsq] land in one [128, 8] SBUF strip so a single
    # group-averaging matmul produces every group's [mean, E[x^2]].
    xg3 = big.tile([128, CT, S], F8, tag="xg3")  # normalized input, [c, s]
    mv8 = gnp.tile([128, 8], F32, tag="mv8", bufs=1)
    for t in range(CT):
        x_t = x_tiles[t]
        sq2 = gnp.tile([128, 2], F32, tag=f"sq2_{t}", name=f"sq2_{t}", bufs=1)
        for h in range(2):
            nc.scalar.activation(
                out=xg3[:, t, h * 2048 : (h + 1) * 2048],
                in_=x_t[:, h * 2048 : (h + 1) * 2048],
                func=Act.Square,
                accum_out=sq2[:, h : h + 1],
            )
        sc = gnp.tile([128, 2048], F16, tag="redsc", name="redsc", bufs=2)
        nc.vector.tensor_add(out=sc[:], in0=x_t[:, 0:2048], in1=x_t[:, 2048:4096])
        nc.vector.tensor_add(out=sc[:, 0:1024], in0=sc[:, 0:1024], in1=sc[:, 1024:2048])
        nc.vector.reduce_sum(
            out=mv8[:, t : t + 1], in_=sc[:, 0:1024], axis=mybir.AxisListType.X
        )
        nc.vector.reduce_sum(
            out=mv8[:, 4 + t : 5 + t], in_=sq2[:], axis=mybir.AxisListType.X
        )
    psg8 = pjsum.tile([128, 8], F32, tag="psg", name="psg8", bufs=1)
    nc.tensor.matmul(psg8[:], lhsT=gmat_sb[:], rhs=mv8[:], start=True, stop=True)
    for r in range(12):
        nc.tensor.matmul(
            wu[:],
            lhsT=ones16[:],
            rhs=x_tiles[r % CT][:, (r * 313) % 3584 : (r * 313) % 3584 + 512],
            start=True,
            stop=True,
        )

    # Pass 2: batched scale/shift chain over all four tiles at once
    # (psg8 = [mean x4 | E[x^2] x4]); scl4/sft4 columns feed the normalize.
    gstat8 = gnp.tile([128, 8], F32, tag="gstat8", bufs=1)
    nc.vector.tensor_copy(out=gstat8[:], in_=psg8[:])
    varg4 = gnp.tile([128, 4], F32, tag="varg4", bufs=1)
    nc.vector.tensor_tensor(
        out=varg4[:], in0=gstat8[:, 0:4], in1=gstat8[:, 0:4], op=mult
    )
    nc.vector.tensor_tensor(out=varg4[:], in0=gstat8[:, 4:8], in1=varg4[:], op=subtract)
    stdt4 = gnp.tile([128, 4], F32, tag="stdt4", bufs=1)
    nc.scalar.activation(out=stdt4[:], in_=varg4[:], func=Act.Sqrt, bias=eps_t[:])
    rstd4 = gnp.tile([128, 4], F32, tag="rstd4", bufs=1)
    nc.vector.reciprocal(out=rstd4[:], in_=stdt4[:])
    scl4 = gnp.tile([128, 4], F32, tag="scl4", bufs=1)
    nc.vector.tensor_tensor(out=scl4[:], in0=rstd4[:], in1=gw_sb[:], op=mult)
    sft4 = gnp.tile([128, 4], F32, tag="sft4", bufs=1)
    nc.vector.tensor_tensor(out=sft4[:], in0=gstat8[:, 0:4], in1=scl4[:], op=mult)
    nc.vector.tensor_tensor(out=sft4[:], in0=gb_sb[:], in1=sft4[:], op=subtract)
    scls = [scl4[:, t : t + 1] for t in range(CT)]
    sfts = [sft4[:, t : t + 1] for t in range(CT)]

    # Pass 3 + projections, token-quarter-major: normalize one 1024-token
    # quarter (all channel tiles), then immediately run the K/V/Q projection
    # pair-groups that consume it. Each pair-group accumulates two 512-token
    # blocks into a 2-bank PSUM tile so the PSUM->SBUF epilogue is a single
    # 1024-wide op, alternating ACT/DVE -- both engines stay under the PE
    # matmul time, making the projection phase PE-bound.
    kt3 = big.tile([128, CT, S], F8, tag="kt3")  # k^T [c, j], x QKSCALE
    qt3 = big.tile([128, CT, SH], F8, tag="qt3")  # q^T [c, i], x QKSCALE
    v3 = big.tile([128, S // 128, C], F8, tag="v3")  # v natural [j, o], x VSCALE
    eng_flip = [0]

    def pair_copy(dst, ps, bias):
        eng_flip[0] ^= 1
        if eng_flip[0]:
            if bias is None:
                nc.scalar.activation(out=dst, in_=ps[:], func=Act.Copy)
            else:
                nc.scalar.activation(out=dst, in_=ps[:], func=Act.Identity, bias=bias)
        else:
            if bias is None:
                nc.vector.tensor_copy(out=dst, in_=ps[:])
            else:
                nc.vector.tensor_scalar(
                    out=dst, in0=ps[:], scalar1=bias, scalar2=None, op0=add
                )

    for qn in range(4):
        q0 = qn * 1024
        for t in range(CT):
            if t == 0:
                nc.scalar.activation(
                    out=xg3[:, t, q0 : q0 + 1024],
                    in_=x_tiles[t][:, q0 : q0 + 1024],
                    func=Act.Identity,
                    bias=sfts[t],
                    scale=scls[t],
                )
            else:
                nc.vector.tensor_scalar(
                    out=xg3[:, t, q0 : q0 + 1024],
                    in0=x_tiles[t][:, q0 : q0 + 1024],
                    scalar1=scls[t],
                    scalar2=sfts[t],
                    op0=mult,
                    op1=add,
                )
        # K: one jc-pair per output tile
        for ot in range(CT):
            ps = pjsum.tile([128, 2, 512], F32, tag="pj", name="ps_k")
            for h in range(2):
                for tp in range(CT // 2):
                    nc.tensor.matmul(
                        ps[:, h, :],
                        lhsT=wk3[:, 2 * tp : 2 * tp + 2, ot * 128 : (ot + 1) * 128],
                        rhs=xg3[:, 2 * tp : 2 * tp + 2, q0 + h * 512 : q0 + (h + 1) * 512],
                        start=(tp == 0),
                        stop=(tp == CT // 2 - 1),
                        perf_mode=DR,
                    )
            pair_copy(kt3[:, ot, q0 : q0 + 1024], ps, bk_sb[:, ot : ot + 1])
        # V: four jb-pairs
        for jbp in range(4):
            jb0 = qn * 8 + 2 * jbp
            ps = pjsum.tile([128, 2, 512], F32, tag="pj", name="ps_v")
            for h in range(2):
                for tp in range(CT // 2):
                    nc.tensor.matmul(
                        ps[:, h, :],
                        lhsT=xg3[:, 2 * tp : 2 * tp + 2, (jb0 + h) * 128 : (jb0 + h + 1) * 128],
                        rhs=wv3[:, 2 * tp : 2 * tp + 2, :],
                        start=(tp == 0),
                        stop=(tp == CT // 2 - 1),
                        perf_mode=DR,
                    )
            pair_copy(v3[:, jb0 : jb0 + 2, :], ps, None)
        # Q: local queries only (token quarters 0 and 1)
        if qn < 2:
            for ot in range(CT):
                ps = pjsum.tile([128, 2, 512], F32, tag="pj", name="ps_q")
                for h in range(2):
                    for tp in range(CT // 2):
                        nc.tensor.matmul(
                            ps[:, h, :],
                            lhsT=wq3[:, 2 * tp : 2 * tp + 2, ot * 128 : (ot + 1) * 128],
                            rhs=xg3[:, 2 * tp : 2 * tp + 2, q0 + h * 512 : q0 + (h + 1) * 512],
                            start=(tp == 0),
                            stop=(tp == CT // 2 - 1),
                            perf_mode=DR,
                        )
                pair_copy(qt3[:, ot, q0 : q0 + 1024], ps, bq_sb[:, ot : ot + 1])

    # release the 6 proj banks, then open the attention PSUM pools:
    # ps pairs (2 banks x 2 bufs) + av0..3 (1 each) = 8 banks. The finisher's
    # denominator/out-proj PSUM shares the "ps" rotation.
    proj_ctx.close()
    ppsum = ctx.enter_context(tc.tile_pool(name="ppsum", bufs=3, space="PSUM"))
    dpsum = ctx.enter_context(tc.tile_pool(name="dpsum", bufs=1, space="PSUM"))
    apsum = ctx.enter_context(tc.tile_pool(name="apsum", bufs=1, space="PSUM"))

    # ---- attention + output projection, per 512-query chunk ----
    # The per-chunk epilogue (denominator, attn-out copies, output projection,
    # residual) is deferred into the next chunk's j-loop so its PE work and
    # PSUM->SBUF copies overlap the next chunk's score matmuls.
    NJP = S // 256  # 16 key-block pairs

    def make_finisher(ic, av, den_ps):
        isl = slice(ic * 512, (ic + 1) * 512)
        state = {}

        def finish_a():
            # PSUM->SBUF attn-out copies gate the next chunk's attnV (av bank
            # reuse): split DVE/GPSIMD so the ACT exp stream is not delayed.
            a4 = smal.tile([128, CT, 512], F8, tag="a4", name="a4")
            for ot in range(CT):
                if ot < 2:
                    nc.vector.tensor_copy(out=a4[:, ot, :], in_=av[ot][:])
                else:
                    nc.scalar.activation(out=a4[:, ot, :], in_=av[ot][:], func=Act.Copy)
            # reciprocal straight off the PE-accumulated denominator bank
            recip = smal.tile([128, 512], F32, tag="recip", name="recip")
            nc.vector.reciprocal(out=recip[:], in_=den_ps[:])
            state["recip"] = recip
            state["a4"] = a4

        def finish_b():
            recip, a4 = state["recip"], state["a4"]
            for ot2 in range(CT):
                osl = slice(ot2 * 128, (ot2 + 1) * 128)
                ps_o = ppsum.tile([128, 512], F32, tag="ps", name="ps_o")
                for tp in range(CT // 2):
                    nc.tensor.matmul(
                        ps_o[:],
                        lhsT=wo3[:, 2 * tp : 2 * tp + 2, osl],
                        rhs=a4[:, 2 * tp : 2 * tp + 2, :],
                        start=(tp == 0),
                        stop=(tp == CT // 2 - 1),
                        perf_mode=DR,
                    )
                res_t = resp.tile([128, 512], F32, tag="res", name="res_t")
                nc.sync.dma_start(out=res_t[:], in_=d["xr"][osl, isl])
                f1 = finp.tile([128, 512], F32, tag="f1", name="f1")
                nc.vector.tensor_tensor(out=f1[:], in0=ps_o[:], in1=recip[:], op=mult)
                nc.vector.scalar_tensor_tensor(
                    out=f1[:],
                    in0=f1[:],
                    scalar=bo_sb[:, ot2 : ot2 + 1],
                    in1=res_t[:],
                    op0=add,
                    op1=add,
                )
                nc.sync.dma_start(out=d["out"][osl, isl], in_=f1[:])

        return finish_a, finish_b

    finish_prev = None
    for ic in range(SH // 512):
        isl = slice(ic * 512, (ic + 1) * 512)
        av = [
            apsum.tile([128, 512], F32, tag=f"av{ot}", name=f"av{ot}")
            for ot in range(CT)
        ]
        den_ps = dpsum.tile([128, 512], F32, tag="den", name="den_ps")
        last_chunk = False
        pe_den = 8
        sums_dve = smal.tile([128, 2, 512], F16, tag="sums_dve", name="sums_dve")
        sums_dvf = smal.tile([128, 512], F16, tag="sums_dvf", name="sums_dvf")

        def scores_exp(jp):
            # scores^T for key blocks 2jp, 2jp+1: two single-bank PSUM tiles,
            # two 512-wide exps into the fp8 pair-layout tile attnV reads.
            e_t = expp.tile([128, 2, 512], F8, tag="exp", name="e_t")
            for h in range(2):
                jb = 2 * jp + h
                ps_s = ppsum.tile([128, 512], F32, tag="ps", name="ps_s")
                for tp in range(CT // 2):
                    nc.tensor.matmul(
                        ps_s[:],
                        lhsT=kt3[:, 2 * tp : 2 * tp + 2, jb * 128 : (jb + 1) * 128],
                        rhs=qt3[:, 2 * tp : 2 * tp + 2, isl],
                        start=(tp == 0),
                        stop=(tp == CT // 2 - 1),
                        perf_mode=DR,
                    )
                nc.scalar.activation(
                    out=e_t[:, h, :],
                    in_=ps_s[:],
                    func=Act.Exp,
                    bias=expb_t[:],
                    scale=EXPSCALE,
                )
            return e_t

        es = {}

        def attnv_sums(jp, e_t):
            for ot in range(CT):
                nc.tensor.matmul(
                    av[ot][:],
                    lhsT=v3[:, 2 * jp : 2 * jp + 2, ot * 128 : (ot + 1) * 128],
                    rhs=e_t[:],
                    start=(jp == 0),
                    stop=(jp == NJP - 1),
                    perf_mode=DR,
                )
            # denominator: pairs 0..7 accumulate on PE (ones-matmul, one pair
            # behind so the previous chunk's reciprocal can read the bank);
            # pairs 8..15 accumulate on DVE in fp16 and join via one final
            # ones-matmul -- splits the cost between the two engines.
            if 0 < jp <= pe_den:
                nc.tensor.matmul(
                    den_ps[:],
                    lhsT=ones8[:],
                    rhs=es[jp - 1][:],
                    start=(jp == 1),
                    stop=(last_chunk and jp == NJP),
                    perf_mode=DR,
                )
            if not last_chunk:
                if jp == 8:
                    nc.vector.tensor_copy(out=sums_dve[:], in_=e_t[:])
                elif jp > 8:
                    nc.vector.tensor_add(out=sums_dve[:], in0=sums_dve[:], in1=e_t[:])

        # Depth-3 score pipeline across the chunk boundary: three pair-groups
        # of scores queue on PE before the first attnV (which must wait for
        # the previous chunk's a4 copies to release the av banks).
        if finish_prev is not None:
            finish_prev[0]()
        es[0] = scores_exp(0)
        es[1] = scores_exp(1)
        es[2] = scores_exp(2)
        es[3] = scores_exp(3)
        es[4] = scores_exp(4)
        es[5] = scores_exp(5)
        for jp in range(NJP):
            attnv_sums(jp, es[jp])
            if jp + 6 < NJP:
                es[jp + 6] = scores_exp(jp + 6)
            if jp == 3 and finish_prev is not None:
                finish_prev[1]()
                finish_prev = None
        if last_chunk:
            nc.tensor.matmul(
                den_ps[:],
                lhsT=ones8[:],
                rhs=es[NJP - 1][:],
                start=False,
                stop=True,
                perf_mode=DR,
            )
        else:
            nc.vector.tensor_add(
                out=sums_dvf[:], in0=sums_dve[:, 0, :], in1=sums_dve[:, 1, :]
            )
            nc.tensor.matmul(
                den_ps[:], lhsT=ones16d[:], rhs=sums_dvf[:], start=False, stop=True
            )
        es.clear()
        finish_prev = make_finisher(ic, av, den_ps)
    finish_prev[0]()
    finish_prev[1]()


_CACHE = {}


def _get_program():
    if "nc" in _CACHE:
        return _CACHE["nc"]
    nc = bacc.Bacc("TRN2", target_bir_lowering=False, debug=False, num_devices=N_CORES)
    d = {}
    d["x"] = nc.dram_tensor("x", [C, S], F16, kind="ExternalInput").ap()
    d["xr"] = nc.dram_tensor("xr", [C, SH], F32, kind="ExternalInput").ap()
    for name in ("wqt", "wkt", "wvt", "wot"):
        d[name] = nc.dram_tensor(name, [C, C], F8, kind="ExternalInput").ap()
    for name in ("bq2", "bk2", "bo2", "gw2", "gb2"):
        d[name] = nc.dram_tensor(name, [128, CT], F32, kind="ExternalInput").ap()
    d["gmat"] = nc.dram_tensor("gmat", [128, 128], F32, kind="ExternalInput").ap()
    d["out"] = nc.dram_tensor("out", [C, SH], F32, kind="ExternalOutput").ap()

    with tile.TileContext(nc) as tc:
        with ExitStack() as ctx:
            _build_kernel(ctx, tc, d)
    nc.compile()
    _CACHE["nc"] = nc
    return nc


def make_in_maps(**inputs):
    """Per-core input dicts (numpy). Core c handles batch c//2, query-half c%2."""
    f32 = np.float32
    hs = np.asarray(inputs["hidden_states"], f32).reshape(B, C, S)
    common = {}
    for wname, key, ws in (
        ("wqt", "wq", QKSCALE),
        ("wkt", "wk", QKSCALE),
        ("wvt", "wv", VSCALE),
        ("wot", "wo", OSCALE),
    ):
        w = np.asarray(inputs[key], f32) * ws
        common[wname] = np.ascontiguousarray(w.T).astype(F8NP)
    # bv folds into the output-projection bias: out += (P @ 1*bv) @ Wo.T
    bo_eff = np.asarray(inputs["bo"], f32) + np.asarray(inputs["wo"], f32) @ np.asarray(
        inputs["bv"], f32
    )
    for bname, bvec, bs in (
        ("bq2", np.asarray(inputs["bq"], f32), QKSCALE),
        ("bk2", np.asarray(inputs["bk"], f32), QKSCALE),
        ("bo2", bo_eff, 1.0),
    ):
        b = bvec * bs
        common[bname] = np.ascontiguousarray(b.reshape(CT, 128).T)
    common["gw2"] = np.ascontiguousarray(
        np.asarray(inputs["gn_weight"], f32).reshape(CT, 128).T
    )
    common["gb2"] = np.ascontiguousarray(
        np.asarray(inputs["gn_bias"], f32).reshape(CT, 128).T
    )
    gmat = np.zeros((128, 128), f32)
    for g in range(128 // GSIZE):
        # averages raw per-partition [sum, sumsq] into per-group [mean, E[x^2]]
        gmat[g * GSIZE : (g + 1) * GSIZE, g * GSIZE : (g + 1) * GSIZE] = 1.0 / (
            GSIZE * S
        )
    common["gmat"] = gmat

    in_maps = []
    for core in range(N_CORES):
        b_idx, half = divmod(core, 2)
        xb = hs[b_idx]
        if half:
            xp = np.concatenate([xb[:, SH:], xb[:, :SH]], axis=1)
        else:
            xp = xb
        m = dict(common)
        m["x"] = np.ascontiguousarray(xp.astype(np.float16))
        m["xr"] = np.ascontiguousarray(xp[:, :SH])
        in_maps.append(m)
    return in_maps


def assemble_output(results):
    out = np.empty((B, C, S), np.float32)
    for core in range(N_CORES):
        b_idx, half = divmod(core, 2)
        out[b_idx][:, half * SH : (half + 1) * SH] = results[core]["out"]
    return out.reshape(B, C, 64, 64)


def run(trace=False, **inputs):
    nc = _get_program()
    in_maps = make_in_maps(**inputs)
    res = run_bass_kernel_spmd(nc, in_maps, core_ids=list(range(N_CORES)), trace=trace)
    return assemble_output(res.results), res


def kernel(**inputs):
    out, _ = run(**inputs)
    return out


# revision 36
# speedup vs baseline: 1.0227x; 1.0227x over previous
"""Trainium2 Bass kernel for an AttentionBlock (GroupNorm -> 1-head attention -> proj -> residual).

Problem: hidden_states (4, 512, 64, 64) fp32; GroupNorm(32 groups) then
single-head attention over S=4096 tokens with head_dim=C=512, output
projection, residual add.

Sharding: 8 cores = 4 batch elements x 2 query-halves. Each core:
 - receives the full [512, 4096] (channels x spatial) slab for its batch
   element, spatially rotated so that *its* 2048 queries are columns 0:2048
   (attention is permutation-invariant over keys, so every core can run the
   identical SPMD program);
 - computes GroupNorm + K/V for all 4096 tokens (redundant x2, cheap) and
   Q only for its half;
 - computes scores^T (keys-on-partition layout), exp, attn @ V, out-proj,
   residual -- no on-chip transposes anywhere.

Numerics: fp8(e4m3) matmul operands with DoubleRow perf mode (two 128-row
k-tiles contracted per PE pass, ~1.9x TensorE throughput over fp16) and fp32
PSUM accumulation. Weights host-pre-scaled (wq,wk x16; wv x4; wo x16) to
keep fp8 operands out of the subnormal range; the scales cancel through the
softmax-denominator constant (64). Softmax without max-subtraction (scores
~ N(0,1)) with a constant exp-bias of -4; normalization deferred past the
output projection. bv and bk fold away algebraically (bv into bo on the
host; k-bias is softmax-invariant but still applied -- it rides the kt copy
for free). Measured end-to-end max-rel error vs fp32 reference: 6.2e-3.

Schedule highlights:
 - PE warmup paced by the x-DMA stream (junk matmuls consuming each arriving
   quarter) keeps HAM at K=8/8 through the GroupNorm latency;
 - GN stats: per-half Square+accum on ACT pipelined with the DMA, fp16
   pairwise-fold sums on DVE, one [128,8] group-averaging matmul, batched
   scale/shift chain;
 - normalize and the K/V/Q projections run token-quarter-major; projection
   epilogues accumulate jc-pairs in 2-bank PSUM tiles so each PSUM->SBUF
   copy is a single 1024-wide op alternating ACT/DVE (projection phase is
   PE-bound);
 - attention: per 512-query chunk, 16 key-block pairs; scores into
   single-bank PSUM tiles (3-deep rotation), exp on ACT into the fp8
   [128,2,512] pair layout attnV's DoubleRow rhs needs; softmax denominator
   accumulated on PE (fp8 ones-matmul, one pair behind) for pairs 0..7 and
   on DVE (fp16) for pairs 8..15 into a dedicated PSUM bank; depth-6 score
   prologue rides chunk boundaries; per-chunk epilogue (reciprocal, attn-out
   copies, out-proj, residual fuse) deferred into the next chunk's loop.

Measured on 8 axon TRN2 cores: ~237us HW exec (~444us for the fp16
non-DoubleRow predecessor); TensorMatrix ~83% busy, ~186us of which is
within ~3% of the DoubleRow MATMUL roofline for the instruction mix.
"""

from contextlib import ExitStack

import ml_dtypes
import numpy as np

import concourse.bacc as bacc
import concourse.tile as tile
from concourse import mybir
from concourse.bass_utils import run_bass_kernel_spmd

F32 = mybir.dt.float32
F16 = mybir.dt.float16
F8 = mybir.dt.float8e4
F8NP = ml_dtypes.float8_e4m3
DR = mybir.MatmulPerfMode.DoubleRow

B = 4
C = 512
S = 4096  # 64*64 tokens
SH = S // 2  # tokens per core (query half)
GROUPS = 32
GSIZE = C // GROUPS  # 16 channels per group
EPS = 1e-6
CT = C // 128  # 4 channel tiles
SCALE = 1.0 / np.sqrt(np.float32(C))
EXPBIAS = -4.0  # constant max-substitute inside exp; cancels in normalization

QKSCALE = 16.0  # host pre-scale on wq/wk/bq/bk (fp8 range use)
VSCALE = 4.0  # host pre-scale on wv/bv (keeps unnormalized attn@V in fp8 range)
OSCALE = 16.0  # host pre-scale on wo
ONES_VAL = VSCALE * OSCALE  # denominator broadcast constant; cancels v/o scales
EXPSCALE = float(SCALE / (QKSCALE * QKSCALE))  # exp() input scale on raw scores

N_CORES = 8


def _build_kernel(ctx: ExitStack, tc: tile.TileContext, d):
    nc = tc.nc
    mult = mybir.AluOpType.mult
    add = mybir.AluOpType.add
    subtract = mybir.AluOpType.subtract
    Act = mybir.ActivationFunctionType

    cst = ctx.enter_context(tc.tile_pool(name="cst", bufs=1))
    xin = ctx.enter_context(tc.tile_pool(name="xin", bufs=3))
    gnp = ctx.enter_context(tc.tile_pool(name="gnp", bufs=4))
    big = ctx.enter_context(tc.tile_pool(name="big", bufs=1))
    expp = ctx.enter_context(tc.tile_pool(name="expp", bufs=8))
    smal = ctx.enter_context(tc.tile_pool(name="smal", bufs=2))
    resp = ctx.enter_context(tc.tile_pool(name="resp", bufs=2))
    finp = ctx.enter_context(tc.tile_pool(name="finp", bufs=2))

    x_d = d["x"]  # fp16 copy of the input slab: GN stats + matmul path
    # sync DMA queue order: channel tile 0 first (it heads the GroupNorm
    # pipeline), then the tiny GN constants it needs, then the other tiles.
    # Four sub-DMAs per tile so bn_stats starts on the first quarter early;
    # each tile gets its own slot so all transfers issue immediately.
    x_tiles = []
    for t in range(CT):
        x_t = xin.tile([128, S], F16, tag=f"xt{t}", name=f"xt{t}", bufs=1)
        x_tiles.append(x_t)

    def dma_x(t):
        for h in range(4):
            nc.sync.dma_start(
                out=x_tiles[t][:, h * (S // 4) : (h + 1) * (S // 4)],
                in_=x_d[t * 128 : (t + 1) * 128, h * (S // 4) : (h + 1) * (S // 4)],
            )

    dma_x(0)
    gmat_raw = cst.tile([128, 128], F32, tag="gmat_raw")
    nc.sync.dma_start(out=gmat_raw[:], in_=d["gmat"][:])
    gw_sb = cst.tile([128, CT], F32, tag="gw")
    nc.sync.dma_start(out=gw_sb[:], in_=d["gw2"][:])
    gb_sb = cst.tile([128, CT], F32, tag="gb")
    nc.sync.dma_start(out=gb_sb[:], in_=d["gb2"][:])
    for t in range(1, CT):
        dma_x(t)

    # ---- constants / weights to SBUF (gpsimd DMA queue; overlaps x).
    # Order = first-use order: K/Q/V weights gate the projections,
    # biases gate the PSUM->SBUF copies a bit later, wo3/bo much later.
    wq3 = cst.tile([128, CT, C], F8, tag="wq3")
    wk3 = cst.tile([128, CT, C], F8, tag="wk3")
    wv3 = cst.tile([128, CT, C], F8, tag="wv3")
    wo3 = cst.tile([128, CT, C], F8, tag="wo3")
    for w_sb, w_d in ((wk3, d["wkt"]), (wq3, d["wqt"]), (wv3, d["wvt"])):
        nc.gpsimd.dma_start(out=w_sb[:], in_=w_d.rearrange("(t p) o -> p t o", p=128))
    bq_sb = cst.tile([128, CT], F32, tag="bq")
    bk_sb = cst.tile([128, CT], F32, tag="bk")
    bo_sb = cst.tile([128, CT], F32, tag="bo")
    for t_sb, t_d in ((bk_sb, d["bk2"]), (bq_sb, d["bq2"]), (bo_sb, d["bo2"])):
        nc.gpsimd.dma_start(out=t_sb[:], in_=t_d[:])
    nc.gpsimd.dma_start(out=wo3[:], in_=d["wot"].rearrange("(t p) o -> p t o", p=128))
    # staging copy: the first PE matmul then depends only on the DVE
    # semaphore (S3_LW allows a single wait)
    gmat_sb = cst.tile([128, 128], F32, tag="gmat")
    nc.vector.tensor_copy(out=gmat_sb[:], in_=gmat_raw[:])
    ones8 = cst.tile([128, 2, 128], F8, tag="ones8")
    nc.vector.memset(ones8[:], float(ONES_VAL))
    ones16 = cst.tile([128, 128], F16, tag="ones16")
    nc.vector.memset(ones16[:], 1.0)
    ones16d = cst.tile([128, 128], F16, tag="ones16d")
    nc.vector.memset(ones16d[:], float(ONES_VAL))
    eps_t = cst.tile([128, 1], F32, tag="epsc")
    nc.vector.memset(eps_t[:], float(EPS))
    expb_t = cst.tile([128, 1], F32, tag="expb")
    nc.vector.memset(expb_t[:], float(EXPBIAS))

    # proj-phase PSUM pool: 6 banks; scoped so its banks are released to the
    # attention pools afterwards
    proj_ctx = ExitStack()
    pjsum = proj_ctx.enter_context(tc.tile_pool(name="pjsum", bufs=3, space="PSUM"))

    # PE warmup, paced by the x DMA: a short front-fill of ones matmuls,
    # then junk matmuls reading each arriving x quarter -- TensorE tracks the
    # transfer stream (HAM stays K=8/8) instead of idling before the first
    # projection.
    wu = pjsum.tile([128, 512], F32, tag="wu", bufs=1)
    for _ in range(55):
        nc.tensor.matmul(
            wu[:, 0:128], lhsT=ones8[:, 0, :], rhs=ones8[:, 0, :], start=True, stop=True
        )
    for t in range(CT):
        for h in range(4):
            for r in range(3):
                nc.tensor.matmul(
                    wu[:],
                    lhsT=ones16[:],
                    rhs=x_tiles[t][:, h * 1024 + r * 170 : h * 1024 + r * 170 + 512],
                    start=True,
                    stop=True,
                )
    # ---- GroupNorm ----
    # Pass 1 (pipelined with the x DMA): per-quarter sum-of-squares on ACT
    # (accum_out; junk main output lands in the xg3 slot, overwritten by the
    # normalize pass) and fp16 pairwise-fold sums on DVE. All four tiles'
    # raw [sum, sumsq] land in one [128, 8] SBUF strip so a single
    # group-averaging matmul produces every group's [mean, E[x^2]].
    xg3 = big.tile([128, CT, S], F8, tag="xg3")  # normalized input, [c, s]
    mv8 = gnp.tile([128, 8], F32, tag="mv8", bufs=1)
    for t in range(CT):
        x_t = x_tiles[t]
        sq2 = gnp.tile([128, 2], F32, tag=f"sq2_{t}", name=f"sq2_{t}", bufs=1)
        for h in range(2):
            nc.scalar.activation(
                out=xg3[:, t, h * 2048 : (h + 1) * 2048],
                in_=x_t[:, h * 2048 : (h + 1) * 2048],
                func=Act.Square,
                accum_out=sq2[:, h : h + 1],
            )
        sc = gnp.tile([128, 2048], F16, tag="redsc", name="redsc", bufs=2)
        nc.vector.tensor_add(out=sc[:], in0=x_t[:, 0:2048], in1=x_t[:, 2048:4096])
        nc.vector.tensor_add(out=sc[:, 0:1024], in0=sc[:, 0:1024], in1=sc[:, 1024:2048])
        nc.vector.reduce_sum(
            out=mv8[:, t : t + 1], in_=sc[:, 0:1024], axis=mybir.AxisListType.X
        )
        nc.vector.reduce_sum(
            out=mv8[:, 4 + t : 5 + t], in_=sq2[:], axis=mybir.AxisListType.X
        )
    psg8 = pjsum.tile([128, 8], F32, tag="psg", name="psg8", bufs=1)
    nc.tensor.matmul(psg8[:], lhsT=gmat_sb[:], rhs=mv8[:], start=True, stop=True)

    # Pass 2: batched scale/shift chain over all four tiles at once
    # (psg8 = [mean x4 | E[x^2] x4]); scl4/sft4 columns feed the normalize.
    gstat8 = gnp.tile([128, 8], F32, tag="gstat8", bufs=1)
    nc.vector.tensor_copy(out=gstat8[:], in_=psg8[:])
    varg4 = gnp.tile([128, 4], F32, tag="varg4", bufs=1)
    nc.vector.tensor_tensor(
        out=varg4[:], in0=gstat8[:, 0:4], in1=gstat8[:, 0:4], op=mult
    )
    nc.vector.tensor_tensor(out=varg4[:], in0=gstat8[:, 4:8], in1=varg4[:], op=subtract)
    stdt4 = gnp.tile([128, 4], F32, tag="stdt4", bufs=1)
    nc.scalar.activation(out=stdt4[:], in_=varg4[:], func=Act.Sqrt, bias=eps_t[:])
    rstd4 = gnp.tile([128, 4], F32, tag="rstd4", bufs=1)
    nc.vector.reciprocal(out=rstd4[:], in_=stdt4[:])
    scl4 = gnp.tile([128, 4], F32, tag="scl4", bufs=1)
    nc.vector.tensor_tensor(out=scl4[:], in0=rstd4[:], in1=gw_sb[:], op=mult)
    sft4 = gnp.tile([128, 4], F32, tag="sft4", bufs=1)
    nc.vector.tensor_tensor(out=sft4[:], in0=gstat8[:, 0:4], in1=scl4[:], op=mult)
    nc.vector.tensor_tensor(out=sft4[:], in0=gb_sb[:], in1=sft4[:], op=subtract)
    scls = [scl4[:, t : t + 1] for t in range(CT)]
    sfts = [sft4[:, t : t + 1] for t in range(CT)]

    # Pass 3 + projections, token-quarter-major: normalize one 1024-token
    # quarter (all channel tiles), then immediately run the K/V/Q projection
    # pair-groups that consume it. Each pair-group accumulates two 512-token
    # blocks into a 2-bank PSUM tile so the PSUM->SBUF epilogue is a single
    # 1024-wide op, alternating ACT/DVE -- both engines stay under the PE
    # matmul time, making the projection phase PE-bound.
    kt3 = big.tile([128, CT, S], F8, tag="kt3")  # k^T [c, j], x QKSCALE
    qt3 = big.tile([128, CT, SH], F8, tag="qt3")  # q^T [c, i], x QKSCALE
    v3 = big.tile([128, S // 128, C], F8, tag="v3")  # v natural [j, o], x VSCALE
    eng_flip = [0]

    def pair_copy(dst, ps, bias):
        eng_flip[0] ^= 1
        if eng_flip[0]:
            if bias is None:
                nc.scalar.activation(out=dst, in_=ps[:], func=Act.Copy)
            else:
                nc.scalar.activation(out=dst, in_=ps[:], func=Act.Identity, bias=bias)
        else:
            if bias is None:
                nc.vector.tensor_copy(out=dst, in_=ps[:])
            else:
                nc.vector.tensor_scalar(
                    out=dst, in0=ps[:], scalar1=bias, scalar2=None, op0=add
                )

    for qn in range(4):
        q0 = qn * 1024
        for t in range(CT):
            if t == 0:
                nc.scalar.activation(
                    out=xg3[:, t, q0 : q0 + 1024],
                    in_=x_tiles[t][:, q0 : q0 + 1024],
                    func=Act.Identity,
                    bias=sfts[t],
                    scale=scls[t],
                )
            else:
                nc.vector.tensor_scalar(
                    out=xg3[:, t, q0 : q0 + 1024],
                    in0=x_tiles[t][:, q0 : q0 + 1024],
                    scalar1=scls[t],
                    scalar2=sfts[t],
                    op0=mult,
                    op1=add,
                )
        # K: one jc-pair per output tile
        for ot in range(CT):
            ps = pjsum.tile([128, 2, 512], F32, tag="pj", name="ps_k")
            for h in range(2):
                for tp in range(CT // 2):
                    nc.tensor.matmul(
                        ps[:, h, :],
                        lhsT=wk3[:, 2 * tp : 2 * tp + 2, ot * 128 : (ot + 1) * 128],
                        rhs=xg3[:, 2 * tp : 2 * tp + 2, q0 + h * 512 : q0 + (h + 1) * 512],
                        start=(tp == 0),
                        stop=(tp == CT // 2 - 1),
                        perf_mode=DR,
                    )
            pair_copy(kt3[:, ot, q0 : q0 + 1024], ps, bk_sb[:, ot : ot + 1])
        # V: four jb-pairs
        for jbp in range(4):
            jb0 = qn * 8 + 2 * jbp
            ps = pjsum.tile([128, 2, 512], F32, tag="pj", name="ps_v")
            for h in range(2):
                for tp in range(CT // 2):
                    nc.tensor.matmul(
                        ps[:, h, :],
                        lhsT=xg3[:, 2 * tp : 2 * tp + 2, (jb0 + h) * 128 : (jb0 + h + 1) * 128],
                        rhs=wv3[:, 2 * tp : 2 * tp + 2, :],
                        start=(tp == 0),
                        stop=(tp == CT // 2 - 1),
                        perf_mode=DR,
                    )
            pair_copy(v3[:, jb0 : jb0 + 2, :], ps, None)
        # Q: local queries only (token quarters 0 and 1)
        if qn < 2:
            for ot in range(CT):
                ps = pjsum.tile([128, 2, 512], F32, tag="pj", name="ps_q")
                for h in range(2):
                    for tp in range(CT // 2):
                        nc.tensor.matmul(
                            ps[:, h, :],
                            lhsT=wq3[:, 2 * tp : 2 * tp + 2, ot * 128 : (ot + 1) * 128],
                            rhs=xg3[:, 2 * tp : 2 * tp + 2, q0 + h * 512 : q0 + (h + 1) * 512],
                            start=(tp == 0),
                            stop=(tp == CT // 2 - 1),
                            perf_mode=DR,
                        )
                pair_copy(qt3[:, ot, q0 : q0 + 1024], ps, bq_sb[:, ot : ot + 1])

    # release the 6 proj banks, then open the attention PSUM pools:
    # ps pairs (2 banks x 2 bufs) + av0..3 (1 each) = 8 banks. The finisher's
    # denominator/out-proj PSUM shares the "ps" rotation.
    proj_ctx.close()
    ppsum = ctx.enter_context(tc.tile_pool(name="ppsum", bufs=3, space="PSUM"))
    dpsum = ctx.enter_context(tc.tile_pool(name="dpsum", bufs=1, space="PSUM"))
    apsum = ctx.enter_context(tc.tile_pool(name="apsum", bufs=1, space="PSUM"))

    # ---- attention + output projection, per 512-query chunk ----
    # The per-chunk epilogue (denominator, attn-out copies, output projection,
    # residual) is deferred into the next chunk's j-loop so its PE work and
    # PSUM->SBUF copies overlap the next chunk's score matmuls.
    NJP = S // 256  # 16 key-block pairs

    def make_finisher(ic, av, den_ps):
        isl = slice(ic * 512, (ic + 1) * 512)
        state = {}

        def finish_a():
            # PSUM->SBUF attn-out copies gate the next chunk's attnV (av bank
            # reuse): split DVE/GPSIMD so the ACT exp stream is not delayed.
            a4 = smal.tile([128, CT, 512], F8, tag="a4", name="a4")
            for ot in range(CT):
                nc.vector.tensor_copy(out=a4[:, ot, :], in_=av[ot][:])
            # reciprocal straight off the PE-accumulated denominator bank
            recip = smal.tile([128, 512], F32, tag="recip", name="recip")
            nc.vector.reciprocal(out=recip[:], in_=den_ps[:])
            state["recip"] = recip
            state["a4"] = a4

        def finish_b():
            recip, a4 = state["recip"], state["a4"]
            for ot2 in range(CT):
                osl = slice(ot2 * 128, (ot2 + 1) * 128)
                ps_o = ppsum.tile([128, 512], F32, tag="ps", name="ps_o")
                for tp in range(CT // 2):
                    nc.tensor.matmul(
                        ps_o[:],
                        lhsT=wo3[:, 2 * tp : 2 * tp + 2, osl],
                        rhs=a4[:, 2 * tp : 2 * tp + 2, :],
                        start=(tp == 0),
                        stop=(tp == CT // 2 - 1),
                        perf_mode=DR,
                    )
                res_t = resp.tile([128, 512], F32, tag="res", name="res_t")
                nc.sync.dma_start(out=res_t[:], in_=d["xr"][osl, isl])
                f1 = finp.tile([128, 512], F32, tag="f1", name="f1")
                nc.vector.tensor_tensor(out=f1[:], in0=ps_o[:], in1=recip[:], op=mult)
                nc.vector.scalar_tensor_tensor(
                    out=f1[:],
                    in0=f1[:],
                    scalar=bo_sb[:, ot2 : ot2 + 1],
                    in1=res_t[:],
                    op0=add,
                    op1=add,
                )
                nc.sync.dma_start(out=d["out"][osl, isl], in_=f1[:])

        return finish_a, finish_b

    finish_prev = None
    for ic in range(SH // 512):
        isl = slice(ic * 512, (ic + 1) * 512)
        av = [
            apsum.tile([128, 512], F32, tag=f"av{ot}", name=f"av{ot}")
            for ot in range(CT)
        ]
        den_ps = dpsum.tile([128, 512], F32, tag="den", name="den_ps")
        last_chunk = False
        pe_den = 8
        sums_dve = smal.tile([128, 2, 512], F16, tag="sums_dve", name="sums_dve")
        sums_dvf = smal.tile([128, 512], F16, tag="sums_dvf", name="sums_dvf")

        def scores_exp(jp):
            # scores^T for key blocks 2jp, 2jp+1: two single-bank PSUM tiles,
            # two 512-wide exps into the fp8 pair-layout tile attnV reads.
            e_t = expp.tile([128, 2, 512], F8, tag="exp", name="e_t")
            for h in range(2):
                jb = 2 * jp + h
                ps_s = ppsum.tile([128, 512], F32, tag="ps", name="ps_s")
                for tp in range(CT // 2):
                    nc.tensor.matmul(
                        ps_s[:],
                        lhsT=kt3[:, 2 * tp : 2 * tp + 2, jb * 128 : (jb + 1) * 128],
                        rhs=qt3[:, 2 * tp : 2 * tp + 2, isl],
                        start=(tp == 0),
                        stop=(tp == CT // 2 - 1),
                        perf_mode=DR,
                    )
                nc.scalar.activation(
                    out=e_t[:, h, :],
                    in_=ps_s[:],
                    func=Act.Exp,
                    bias=expb_t[:],
                    scale=EXPSCALE,
                )
            return e_t

        es = {}

        def attnv_sums(jp, e_t):
            for ot in range(CT):
                nc.tensor.matmul(
                    av[ot][:],
                    lhsT=v3[:, 2 * jp : 2 * jp + 2, ot * 128 : (ot + 1) * 128],
                    rhs=e_t[:],
                    start=(jp == 0),
                    stop=(jp == NJP - 1),
                    perf_mode=DR,
                )
            # denominator: pairs 0..7 accumulate on PE (ones-matmul, one pair
            # behind so the previous chunk's reciprocal can read the bank);
            # pairs 8..15 accumulate on DVE in fp16 and join via one final
            # ones-matmul -- splits the cost between the two engines.
            if 0 < jp <= pe_den:
                nc.tensor.matmul(
                    den_ps[:],
                    lhsT=ones8[:],
                    rhs=es[jp - 1][:],
                    start=(jp == 1),
                    stop=(last_chunk and jp == NJP),
                    perf_mode=DR,
                )
            if not last_chunk:
                if jp == 8:
                    nc.vector.tensor_copy(out=sums_dve[:], in_=e_t[:])
                elif jp > 8:
                    nc.vector.tensor_add(out=sums_dve[:], in0=sums_dve[:], in1=e_t[:])

        # Depth-3 score pipeline across the chunk boundary: three pair-groups
        # of scores queue on PE before the first attnV (which must wait for
        # the previous chunk's a4 copies to release the av banks).
        es[0] = scores_exp(0)
        es[1] = scores_exp(1)
        if finish_prev is not None:
            finish_prev[0]()
        es[2] = scores_exp(2)
        es[3] = scores_exp(3)
        es[4] = scores_exp(4)
        es[5] = scores_exp(5)
        for jp in range(NJP):
            attnv_sums(jp, es[jp])
            if jp + 6 < NJP:
                es[jp + 6] = scores_exp(jp + 6)
            if jp == 3 and finish_prev is not None:
                finish_prev[1]()
                finish_prev = None
        if last_chunk:
            nc.tensor.matmul(
                den_ps[:],
                lhsT=ones8[:],
                rhs=es[NJP - 1][:],
                start=False,
                stop=True,
                perf_mode=DR,
            )
        else:
            nc.vector.tensor_add(
                out=sums_dvf[:], in0=sums_dve[:, 0, :], in1=sums_dve[:, 1, :]
            )
            nc.tensor.matmul(
                den_ps[:], lhsT=ones16d[:], rhs=sums_dvf[:], start=False, stop=True
            )
        es.clear()
        finish_prev = make_finisher(ic, av, den_ps)
    finish_prev[0]()
    finish_prev[1]()


_CACHE = {}


def _get_program():
    if "nc" in _CACHE:
        return _CACHE["nc"]
    nc = bacc.Bacc("TRN2", target_bir_lowering=False, debug=False, num_devices=N_CORES)
    d = {}
    d["x"] = nc.dram_tensor("x", [C, S], F16, kind="ExternalInput").ap()
    d["xr"] = nc.dram_tensor("xr", [C, SH], F32, kind="ExternalInput").ap()
    for name in ("wqt", "wkt", "wvt", "wot"):
        d[name] = nc.dram_tensor(name, [C, C], F8, kind="ExternalInput").ap()
    for name in ("bq2", "bk2", "bo2", "gw2", "gb2"):
        d[name] = nc.dram_tensor(name, [128, CT], F32, kind="ExternalInput").ap()
    d["gmat"] = nc.dram_tensor("gmat", [128, 128], F32, kind="ExternalInput").ap()
    d["out"] = nc.dram_tensor("out", [C, SH], F32, kind="ExternalOutput").ap()

    with tile.TileContext(nc) as tc:
        with ExitStack() as ctx:
            _build_kernel(ctx, tc, d)
    nc.compile()
    _CACHE["nc"] = nc
    return nc


def make_in_maps(**inputs):
    """Per-core input dicts (numpy). Core c handles batch c//2, query-half c%2."""
    f32 = np.float32
    hs = np.asarray(inputs["hidden_states"], f32).reshape(B, C, S)
    common = {}
    for wname, key, ws in (
        ("wqt", "wq", QKSCALE),
        ("wkt", "wk", QKSCALE),
        ("wvt", "wv", VSCALE),
        ("wot", "wo", OSCALE),
    ):
        w = np.asarray(inputs[key], f32) * ws
        common[wname] = np.ascontiguousarray(w.T).astype(F8NP)
    # bv folds into the output-projection bias: out += (P @ 1*bv) @ Wo.T
    bo_eff = np.asarray(inputs["bo"], f32) + np.asarray(inputs["wo"], f32) @ np.asarray(
        inputs["bv"], f32
    )
    for bname, bvec, bs in (
        ("bq2", np.asarray(inputs["bq"], f32), QKSCALE),
        ("bk2", np.asarray(inputs["bk"], f32), QKSCALE),
        ("bo2", bo_eff, 1.0),
    ):
        b = bvec * bs
        common[bname] = np.ascontiguousarray(b.reshape(CT, 128).T)
    common["gw2"] = np.ascontiguousarray(
        np.asarray(inputs["gn_weight"], f32).reshape(CT, 128).T
    )
    common["gb2"] = np.ascontiguousarray(
        np.asarray(inputs["gn_bias"], f32).reshape(CT, 128).T
    )
    gmat = np.zeros((128, 128), f32)
    for g in range(128 // GSIZE):
        # averages raw per-partition [sum, sumsq] into per-group [mean, E[x^2]]
        gmat[g * GSIZE : (g + 1) * GSIZE, g * GSIZE : (g + 1) * GSIZE] = 1.0 / (
            GSIZE * S
        )
    common["gmat"] = gmat

    in_maps = []
    for core in range(N_CORES):
        b_idx, half = divmod(core, 2)
        xb = hs[b_idx]
        if half:
            xp = np.concatenate([xb[:, SH:], xb[:, :SH]], axis=1)
        else:
            xp = xb
        m = dict(common)
        m["x"] = np.ascontiguousarray(xp.astype(np.float16))
        m["xr"] = np.ascontiguousarray(xp[:, :SH])
        in_maps.append(m)
    return in_maps


def assemble_output(results):
    out = np.empty((B, C, S), np.float32)
    for core in range(N_CORES):
        b_idx, half = divmod(core, 2)
        out[b_idx][:, half * SH : (half + 1) * SH] = results[core]["out"]
    return out.reshape(B, C, 64, 64)


def run(trace=False, **inputs):
    nc = _get_program()
    in_maps = make_in_maps(**inputs)
    res = run_bass_kernel_spmd(nc, in_maps, core_ids=list(range(N_CORES)), trace=trace)
    return assemble_output(res.results), res


def kernel(**inputs):
    out, _ = run(**inputs)
    return out


# revision 37
# speedup vs baseline: 1.0393x; 1.0163x over previous
"""Trainium2 Bass kernel for an AttentionBlock (GroupNorm -> 1-head attention -> proj -> residual).

Problem: hidden_states (4, 512, 64, 64) fp32; GroupNorm(32 groups) then
single-head attention over S=4096 tokens with head_dim=C=512, output
projection, residual add.

Sharding: 8 cores = 4 batch elements x 2 query-halves. Each core:
 - receives the full [512, 4096] (channels x spatial) slab for its batch
   element, spatially rotated so that *its* 2048 queries are columns 0:2048
   (attention is permutation-invariant over keys, so every core can run the
   identical SPMD program);
 - computes GroupNorm + K/V for all 4096 tokens (redundant x2, cheap) and
   Q only for its half;
 - computes scores^T (keys-on-partition layout), exp, attn @ V, out-proj,
   residual -- no on-chip transposes anywhere.

Numerics: fp8(e4m3) matmul operands with DoubleRow perf mode (two 128-row
k-tiles contracted per PE pass, ~1.9x TensorE throughput over fp16) and fp32
PSUM accumulation. Weights host-pre-scaled (wq,wk x16; wv x4; wo x16) to
keep fp8 operands out of the subnormal range; the scales cancel through the
softmax-denominator constant (64). Softmax without max-subtraction (scores
~ N(0,1)) with a constant exp-bias of -4; normalization deferred past the
output projection. bv and bk fold away algebraically (bv into bo on the
host; k-bias is softmax-invariant but still applied -- it rides the kt copy
for free). Measured end-to-end max-rel error vs fp32 reference: 6.2e-3.

Schedule highlights:
 - PE warmup paced by the x-DMA stream (junk matmuls consuming each arriving
   quarter) keeps HAM at K=8/8 through the GroupNorm latency;
 - GN stats: per-half Square+accum on ACT pipelined with the DMA, fp16
   pairwise-fold sums on DVE, one [128,8] group-averaging matmul, batched
   scale/shift chain;
 - normalize and the K/V/Q projections run token-quarter-major; projection
   epilogues accumulate jc-pairs in 2-bank PSUM tiles so each PSUM->SBUF
   copy is a single 1024-wide op alternating ACT/DVE (projection phase is
   PE-bound);
 - attention: per 512-query chunk, 16 key-block pairs; scores into
   single-bank PSUM tiles (3-deep rotation), exp on ACT into the fp8
   [128,2,512] pair layout attnV's DoubleRow rhs needs; softmax denominator
   accumulated on PE (fp8 ones-matmul, one pair behind) for pairs 0..7 and
   on DVE (fp16) for pairs 8..15 into a dedicated PSUM bank; depth-6 score
   prologue rides chunk boundaries; per-chunk epilogue (reciprocal, attn-out
   copies, out-proj, residual fuse) deferred into the next chunk's loop.

Measured on 8 axon TRN2 cores: ~237us HW exec (~444us for the fp16
non-DoubleRow predecessor); TensorMatrix ~83% busy, ~186us of which is
within ~3% of the DoubleRow MATMUL roofline for the instruction mix.
"""

from contextlib import ExitStack

import ml_dtypes
import numpy as np

import concourse.bacc as bacc
import concourse.tile as tile
from concourse import mybir
from concourse.bass_utils import run_bass_kernel_spmd

F32 = mybir.dt.float32
F16 = mybir.dt.float16
F8 = mybir.dt.float8e4
F8NP = ml_dtypes.float8_e4m3
DR = mybir.MatmulPerfMode.DoubleRow

B = 4
C = 512
S = 4096  # 64*64 tokens
SH = S // 2  # tokens per core (query half)
GROUPS = 32
GSIZE = C // GROUPS  # 16 channels per group
EPS = 1e-6
CT = C // 128  # 4 channel tiles
SCALE = 1.0 / np.sqrt(np.float32(C))
EXPBIAS = -4.0  # constant max-substitute inside exp; cancels in normalization

QKSCALE = 16.0  # host pre-scale on wq/wk/bq/bk (fp8 range use)
VSCALE = 4.0  # host pre-scale on wv/bv (keeps unnormalized attn@V in fp8 range)
OSCALE = 16.0  # host pre-scale on wo
ONES_VAL = VSCALE * OSCALE  # denominator broadcast constant; cancels v/o scales
EXPSCALE = float(SCALE / (QKSCALE * QKSCALE))  # exp() input scale on raw scores

N_CORES = 8


def _build_kernel(ctx: ExitStack, tc: tile.TileContext, d):
    nc = tc.nc
    mult = mybir.AluOpType.mult
    add = mybir.AluOpType.add
    subtract = mybir.AluOpType.subtract
    Act = mybir.ActivationFunctionType

    cst = ctx.enter_context(tc.tile_pool(name="cst", bufs=1))
    xin = ctx.enter_context(tc.tile_pool(name="xin", bufs=3))
    gnp = ctx.enter_context(tc.tile_pool(name="gnp", bufs=4))
    big = ctx.enter_context(tc.tile_pool(name="big", bufs=1))
    expp = ctx.enter_context(tc.tile_pool(name="expp", bufs=8))
    smal = ctx.enter_context(tc.tile_pool(name="smal", bufs=2))
    resp = ctx.enter_context(tc.tile_pool(name="resp", bufs=2))
    finp = ctx.enter_context(tc.tile_pool(name="finp", bufs=2))

    x_d = d["x"]  # fp16 copy of the input slab: GN stats + matmul path
    # sync DMA queue order: channel tile 0 first (it heads the GroupNorm
    # pipeline), then the tiny GN constants it needs, then the other tiles.
    # Four sub-DMAs per tile so bn_stats starts on the first quarter early;
    # each tile gets its own slot so all transfers issue immediately.
    x_tiles = []
    for t in range(CT):
        x_t = xin.tile([128, S], F16, tag=f"xt{t}", name=f"xt{t}", bufs=1)
        x_tiles.append(x_t)

    def dma_x(t):
        for h in range(4):
            nc.sync.dma_start(
                out=x_tiles[t][:, h * (S // 4) : (h + 1) * (S // 4)],
                in_=x_d[t * 128 : (t + 1) * 128, h * (S // 4) : (h + 1) * (S // 4)],
            )

    dma_x(0)
    gmat_raw = cst.tile([128, 128], F32, tag="gmat_raw")
    nc.sync.dma_start(out=gmat_raw[:], in_=d["gmat"][:])
    gw_sb = cst.tile([128, CT], F32, tag="gw")
    nc.sync.dma_start(out=gw_sb[:], in_=d["gw2"][:])
    gb_sb = cst.tile([128, CT], F32, tag="gb")
    nc.sync.dma_start(out=gb_sb[:], in_=d["gb2"][:])
    for t in range(1, CT):
        dma_x(t)

    # ---- constants / weights to SBUF (gpsimd DMA queue; overlaps x).
    # Order = first-use order: K/Q/V weights gate the projections,
    # biases gate the PSUM->SBUF copies a bit later, wo3/bo much later.
    wq3 = cst.tile([128, CT, C], F8, tag="wq3")
    wk3 = cst.tile([128, CT, C], F8, tag="wk3")
    wv3 = cst.tile([128, CT, C], F8, tag="wv3")
    wo3 = cst.tile([128, CT, C], F8, tag="wo3")
    for w_sb, w_d in ((wk3, d["wkt"]), (wq3, d["wqt"]), (wv3, d["wvt"])):
        nc.gpsimd.dma_start(out=w_sb[:], in_=w_d.rearrange("(t p) o -> p t o", p=128))
    bq_sb = cst.tile([128, CT], F32, tag="bq")
    bk_sb = cst.tile([128, CT], F32, tag="bk")
    bo_sb = cst.tile([128, CT], F32, tag="bo")
    for t_sb, t_d in ((bk_sb, d["bk2"]), (bq_sb, d["bq2"]), (bo_sb, d["bo2"])):
        nc.gpsimd.dma_start(out=t_sb[:], in_=t_d[:])
    nc.gpsimd.dma_start(out=wo3[:], in_=d["wot"].rearrange("(t p) o -> p t o", p=128))
    # staging copy: the first PE matmul then depends only on the DVE
    # semaphore (S3_LW allows a single wait)
    gmat_sb = cst.tile([128, 128], F32, tag="gmat")
    nc.vector.tensor_copy(out=gmat_sb[:], in_=gmat_raw[:])
    ones8 = cst.tile([128, 2, 128], F8, tag="ones8")
    nc.vector.memset(ones8[:], float(ONES_VAL))
    ones16 = cst.tile([128, 128], F16, tag="ones16")
    nc.vector.memset(ones16[:], 1.0)
    ones16d = cst.tile([128, 128], F16, tag="ones16d")
    nc.vector.memset(ones16d[:], float(ONES_VAL))
    eps_t = cst.tile([128, 1], F32, tag="epsc")
    nc.vector.memset(eps_t[:], float(EPS))
    expb_t = cst.tile([128, 1], F32, tag="expb")
    nc.vector.memset(expb_t[:], float(EXPBIAS))

    # proj-phase PSUM pool: 6 banks; scoped so its banks are released to the
    # attention pools afterwards
    proj_ctx = ExitStack()
    pjsum = proj_ctx.enter_context(tc.tile_pool(name="pjsum", bufs=3, space="PSUM"))

    # PE warmup, paced by the x DMA: a short front-fill of ones matmuls,
    # then junk matmuls reading each arriving x quarter -- TensorE tracks the
    # transfer stream (HAM stays K=8/8) instead of idling before the first
    # projection.
    wu = pjsum.tile([128, 512], F32, tag="wu", bufs=1)
    for _ in range(55):
        nc.tensor.matmul(
            wu[:, 0:128], lhsT=ones8[:, 0, :], rhs=ones8[:, 0, :], start=True, stop=True
        )
    for t in range(CT):
        for h in range(4):
            for r in range(3):
                nc.tensor.matmul(
                    wu[:],
                    lhsT=ones16[:],
                    rhs=x_tiles[t][:, h * 1024 + r * 170 : h * 1024 + r * 170 + 512],
                    start=True,
                    stop=True,
                )
    # ---- GroupNorm ----
    # Pass 1 (pipelined with the x DMA): per-quarter sum-of-squares on ACT
    # (accum_out; junk main output lands in the xg3 slot, overwritten by the
    # normalize pass) and fp16 pairwise-fold sums on DVE. All four tiles'
    # raw [sum, sumsq] land in one [128, 8] SBUF strip so a single
    # group-averaging matmul produces every group's [mean, E[x^2]].
    xg3 = big.tile([128, CT, S], F8, tag="xg3")  # normalized input, [c, s]
    mv8 = gnp.tile([128, 8], F32, tag="mv8", bufs=1)
    for t in range(CT):
        x_t = x_tiles[t]
        sq2 = gnp.tile([128, 2], F32, tag=f"sq2_{t}", name=f"sq2_{t}", bufs=1)
        for h in range(2):
            nc.scalar.activation(
                out=xg3[:, t, h * 2048 : (h + 1) * 2048],
                in_=x_t[:, h * 2048 : (h + 1) * 2048],
                func=Act.Square,
                accum_out=sq2[:, h : h + 1],
            )
        sc = gnp.tile([128, 2048], F16, tag="redsc", name="redsc", bufs=2)
        nc.vector.tensor_add(out=sc[:], in0=x_t[:, 0:2048], in1=x_t[:, 2048:4096])
        nc.vector.tensor_add(out=sc[:, 0:1024], in0=sc[:, 0:1024], in1=sc[:, 1024:2048])
        nc.vector.reduce_sum(
            out=mv8[:, t : t + 1], in_=sc[:, 0:1024], axis=mybir.AxisListType.X
        )
        nc.vector.reduce_sum(
            out=mv8[:, 4 + t : 5 + t], in_=sq2[:], axis=mybir.AxisListType.X
        )
    psg8 = pjsum.tile([128, 8], F32, tag="psg", name="psg8", bufs=1)
    nc.tensor.matmul(psg8[:], lhsT=gmat_sb[:], rhs=mv8[:], start=True, stop=True)

    # Pass 2: batched scale/shift chain over all four tiles at once
    # (psg8 = [mean x4 | E[x^2] x4]); scl4/sft4 columns feed the normalize.
    gstat8 = gnp.tile([128, 8], F32, tag="gstat8", bufs=1)
    nc.vector.tensor_copy(out=gstat8[:], in_=psg8[:])
    varg4 = gnp.tile([128, 4], F32, tag="varg4", bufs=1)
    nc.vector.tensor_tensor(
        out=varg4[:], in0=gstat8[:, 0:4], in1=gstat8[:, 0:4], op=mult
    )
    nc.vector.tensor_tensor(out=varg4[:], in0=gstat8[:, 4:8], in1=varg4[:], op=subtract)
    stdt4 = gnp.tile([128, 4], F32, tag="stdt4", bufs=1)
    nc.scalar.activation(out=stdt4[:], in_=varg4[:], func=Act.Sqrt, bias=eps_t[:])
    rstd4 = gnp.tile([128, 4], F32, tag="rstd4", bufs=1)
    nc.vector.reciprocal(out=rstd4[:], in_=stdt4[:])
    scl4 = gnp.tile([128, 4], F32, tag="scl4", bufs=1)
    nc.vector.tensor_tensor(out=scl4[:], in0=rstd4[:], in1=gw_sb[:], op=mult)
    sft4 = gnp.tile([128, 4], F32, tag="sft4", bufs=1)
    nc.vector.tensor_tensor(out=sft4[:], in0=gstat8[:, 0:4], in1=scl4[:], op=mult)
    nc.vector.tensor_tensor(out=sft4[:], in0=gb_sb[:], in1=sft4[:], op=subtract)
    scls = [scl4[:, t : t + 1] for t in range(CT)]
    sfts = [sft4[:, t : t + 1] for t in range(CT)]

    # Pass 3 + projections, token-quarter-major: normalize one 1024-token
    # quarter (all channel tiles), then immediately run the K/V/Q projection
    # pair-groups that consume it. Each pair-group accumulates two 512-token
    # blocks into a 2-bank PSUM tile so the PSUM->SBUF epilogue is a single
    # 1024-wide op, alternating ACT/DVE -- both engines stay under the PE
    # matmul time, making the projection phase PE-bound.
    kt3 = big.tile([128, CT, S], F8, tag="kt3")  # k^T [c, j], x QKSCALE
    qt3 = big.tile([128, CT, SH], F8, tag="qt3")  # q^T [c, i], x QKSCALE
    v3 = big.tile([128, S // 128, C], F8, tag="v3")  # v natural [j, o], x VSCALE
    eng_flip = [0]

    def pair_copy(dst, ps, bias):
        eng_flip[0] ^= 1
        if eng_flip[0]:
            if bias is None:
                nc.scalar.activation(out=dst, in_=ps[:], func=Act.Copy)
            else:
                nc.scalar.activation(out=dst, in_=ps[:], func=Act.Identity, bias=bias)
        else:
            if bias is None:
                nc.vector.tensor_copy(out=dst, in_=ps[:])
            else:
                nc.vector.tensor_scalar(
                    out=dst, in0=ps[:], scalar1=bias, scalar2=None, op0=add
                )

    for qn in range(4):
        q0 = qn * 1024
        for t in range(CT):
            if t < 2:
                nc.scalar.activation(
                    out=xg3[:, t, q0 : q0 + 1024],
                    in_=x_tiles[t][:, q0 : q0 + 1024],
                    func=Act.Identity,
                    bias=sfts[t],
                    scale=scls[t],
                )
            else:
                nc.vector.tensor_scalar(
                    out=xg3[:, t, q0 : q0 + 1024],
                    in0=x_tiles[t][:, q0 : q0 + 1024],
                    scalar1=scls[t],
                    scalar2=sfts[t],
                    op0=mult,
                    op1=add,
                )
        # K: one jc-pair per output tile
        for ot in range(CT):
            ps = pjsum.tile([128, 2, 512], F32, tag="pj", name="ps_k")
            for h in range(2):
                for tp in range(CT // 2):
                    nc.tensor.matmul(
                        ps[:, h, :],
                        lhsT=wk3[:, 2 * tp : 2 * tp + 2, ot * 128 : (ot + 1) * 128],
                        rhs=xg3[:, 2 * tp : 2 * tp + 2, q0 + h * 512 : q0 + (h + 1) * 512],
                        start=(tp == 0),
                        stop=(tp == CT // 2 - 1),
                        perf_mode=DR,
                    )
            pair_copy(kt3[:, ot, q0 : q0 + 1024], ps, bk_sb[:, ot : ot + 1])
        # V: four jb-pairs
        for jbp in range(4):
            jb0 = qn * 8 + 2 * jbp
            ps = pjsum.tile([128, 2, 512], F32, tag="pj", name="ps_v")
            for h in range(2):
                for tp in range(CT // 2):
                    nc.tensor.matmul(
                        ps[:, h, :],
                        lhsT=xg3[:, 2 * tp : 2 * tp + 2, (jb0 + h) * 128 : (jb0 + h + 1) * 128],
                        rhs=wv3[:, 2 * tp : 2 * tp + 2, :],
                        start=(tp == 0),
                        stop=(tp == CT // 2 - 1),
                        perf_mode=DR,
                    )
            pair_copy(v3[:, jb0 : jb0 + 2, :], ps, None)
        # Q: local queries only (token quarters 0 and 1)
        if qn < 2:
            for ot in range(CT):
                ps = pjsum.tile([128, 2, 512], F32, tag="pj", name="ps_q")
                for h in range(2):
                    for tp in range(CT // 2):
                        nc.tensor.matmul(
                            ps[:, h, :],
                            lhsT=wq3[:, 2 * tp : 2 * tp + 2, ot * 128 : (ot + 1) * 128],
                            rhs=xg3[:, 2 * tp : 2 * tp + 2, q0 + h * 512 : q0 + (h + 1) * 512],
                            start=(tp == 0),
                            stop=(tp == CT // 2 - 1),
                            perf_mode=DR,
                        )
                pair_copy(qt3[:, ot, q0 : q0 + 1024], ps, bq_sb[:, ot : ot + 1])

    # release the 6 proj banks, then open the attention PSUM pools:
    # ps pairs (2 banks x 2 bufs) + av0..3 (1 each) = 8 banks. The finisher's
    # denominator/out-proj PSUM shares the "ps" rotation.
    proj_ctx.close()
    ppsum = ctx.enter_context(tc.tile_pool(name="ppsum", bufs=3, space="PSUM"))
    dpsum = ctx.enter_context(tc.tile_pool(name="dpsum", bufs=1, space="PSUM"))
    apsum = ctx.enter_context(tc.tile_pool(name="apsum", bufs=1, space="PSUM"))

    # ---- attention + output projection, per 512-query chunk ----
    # The per-chunk epilogue (denominator, attn-out copies, output projection,
    # residual) is deferred into the next chunk's j-loop so its PE work and
    # PSUM->SBUF copies overlap the next chunk's score matmuls.
    NJP = S // 256  # 16 key-block pairs

    def make_finisher(ic, av, den_ps):
        isl = slice(ic * 512, (ic + 1) * 512)
        tail_split = ic == SH // 512 - 1
        state = {}

        def finish_a():
            # PSUM->SBUF attn-out copies gate the next chunk's attnV (av bank
            # reuse): split DVE/GPSIMD so the ACT exp stream is not delayed.
            a4 = smal.tile([128, CT, 512], F8, tag="a4", name="a4")
            for ot in range(CT):
                if tail_split and ot >= 2:
                    nc.scalar.activation(out=a4[:, ot, :], in_=av[ot][:], func=Act.Copy)
                else:
                    nc.vector.tensor_copy(out=a4[:, ot, :], in_=av[ot][:])
            # reciprocal straight off the PE-accumulated denominator bank
            recip = smal.tile([128, 512], F32, tag="recip", name="recip")
            nc.vector.reciprocal(out=recip[:], in_=den_ps[:])
            state["recip"] = recip
            state["a4"] = a4

        def finish_b():
            recip, a4 = state["recip"], state["a4"]
            for ot2 in range(CT):
                osl = slice(ot2 * 128, (ot2 + 1) * 128)
                ps_o = ppsum.tile([128, 512], F32, tag="ps", name="ps_o")
                for tp in range(CT // 2):
                    nc.tensor.matmul(
                        ps_o[:],
                        lhsT=wo3[:, 2 * tp : 2 * tp + 2, osl],
                        rhs=a4[:, 2 * tp : 2 * tp + 2, :],
                        start=(tp == 0),
                        stop=(tp == CT // 2 - 1),
                        perf_mode=DR,
                    )
                res_t = resp.tile([128, 512], F32, tag="res", name="res_t")
                nc.sync.dma_start(out=res_t[:], in_=d["xr"][osl, isl])
                f1 = finp.tile([128, 512], F32, tag="f1", name="f1")
                nc.vector.tensor_tensor(out=f1[:], in0=ps_o[:], in1=recip[:], op=mult)
                nc.vector.scalar_tensor_tensor(
                    out=f1[:],
                    in0=f1[:],
                    scalar=bo_sb[:, ot2 : ot2 + 1],
                    in1=res_t[:],
                    op0=add,
                    op1=add,
                )
                nc.sync.dma_start(out=d["out"][osl, isl], in_=f1[:])

        return finish_a, finish_b

    finish_prev = None
    for ic in range(SH // 512):
        isl = slice(ic * 512, (ic + 1) * 512)
        av = [
            apsum.tile([128, 512], F32, tag=f"av{ot}", name=f"av{ot}")
            for ot in range(CT)
        ]
        den_ps = dpsum.tile([128, 512], F32, tag="den", name="den_ps")
        last_chunk = False
        pe_den = 8
        sums_dve = smal.tile([128, 2, 512], F16, tag="sums_dve", name="sums_dve")
        sums_dvf = smal.tile([128, 512], F16, tag="sums_dvf", name="sums_dvf")

        def scores_exp(jp):
            # scores^T for key blocks 2jp, 2jp+1: two single-bank PSUM tiles,
            # two 512-wide exps into the fp8 pair-layout tile attnV reads.
            e_t = expp.tile([128, 2, 512], F8, tag="exp", name="e_t")
            for h in range(2):
                jb = 2 * jp + h
                ps_s = ppsum.tile([128, 512], F32, tag="ps", name="ps_s")
                for tp in range(CT // 2):
                    nc.tensor.matmul(
                        ps_s[:],
                        lhsT=kt3[:, 2 * tp : 2 * tp + 2, jb * 128 : (jb + 1) * 128],
                        rhs=qt3[:, 2 * tp : 2 * tp + 2, isl],
                        start=(tp == 0),
                        stop=(tp == CT // 2 - 1),
                        perf_mode=DR,
                    )
                nc.scalar.activation(
                    out=e_t[:, h, :],
                    in_=ps_s[:],
                    func=Act.Exp,
                    bias=expb_t[:],
                    scale=EXPSCALE,
                )
            return e_t

        es = {}

        def attnv_sums(jp, e_t):
            for ot in range(CT):
                nc.tensor.matmul(
                    av[ot][:],
                    lhsT=v3[:, 2 * jp : 2 * jp + 2, ot * 128 : (ot + 1) * 128],
                    rhs=e_t[:],
                    start=(jp == 0),
                    stop=(jp == NJP - 1),
                    perf_mode=DR,
                )
            # denominator: pairs 0..7 accumulate on PE (ones-matmul, one pair
            # behind so the previous chunk's reciprocal can read the bank);
            # pairs 8..15 accumulate on DVE in fp16 and join via one final
            # ones-matmul -- splits the cost between the two engines.
            if 0 < jp <= pe_den:
                nc.tensor.matmul(
                    den_ps[:],
                    lhsT=ones8[:],
                    rhs=es[jp - 1][:],
                    start=(jp == 1),
                    stop=(last_chunk and jp == NJP),
                    perf_mode=DR,
                )
            if not last_chunk:
                if jp == 8:
                    nc.vector.tensor_copy(out=sums_dve[:], in_=e_t[:])
                elif jp > 8:
                    nc.vector.tensor_add(out=sums_dve[:], in0=sums_dve[:], in1=e_t[:])

        # Depth-3 score pipeline across the chunk boundary: three pair-groups
        # of scores queue on PE before the first attnV (which must wait for
        # the previous chunk's a4 copies to release the av banks).
        es[0] = scores_exp(0)
        es[1] = scores_exp(1)
        if finish_prev is not None:
            finish_prev[0]()
        es[2] = scores_exp(2)
        es[3] = scores_exp(3)
        es[4] = scores_exp(4)
        es[5] = scores_exp(5)
        for jp in range(NJP):
            attnv_sums(jp, es[jp])
            if jp + 6 < NJP:
                es[jp + 6] = scores_exp(jp + 6)
            if jp == 3 and finish_prev is not None:
                finish_prev[1]()
                finish_prev = None
        if last_chunk:
            nc.tensor.matmul(
                den_ps[:],
                lhsT=ones8[:],
                rhs=es[NJP - 1][:],
                start=False,
                stop=True,
                perf_mode=DR,
            )
        else:
            nc.vector.tensor_add(
                out=sums_dvf[:], in0=sums_dve[:, 0, :], in1=sums_dve[:, 1, :]
            )
            nc.tensor.matmul(
                den_ps[:], lhsT=ones16d[:], rhs=sums_dvf[:], start=False, stop=True
            )
        es.clear()
        finish_prev = make_finisher(ic, av, den_ps)
    finish_prev[0]()
    finish_prev[1]()


_CACHE = {}


def _get_program():
    if "nc" in _CACHE:
        return _CACHE["nc"]
    nc = bacc.Bacc("TRN2", target_bir_lowering=False, debug=False, num_devices=N_CORES)
    d = {}
    d["x"] = nc.dram_tensor("x", [C, S], F16, kind="ExternalInput").ap()
    d["xr"] = nc.dram_tensor("xr", [C, SH], F32, kind="ExternalInput").ap()
    for name in ("wqt", "wkt", "wvt", "wot"):
        d[name] = nc.dram_tensor(name, [C, C], F8, kind="ExternalInput").ap()
    for name in ("bq2", "bk2", "bo2", "gw2", "gb2"):
        d[name] = nc.dram_tensor(name, [128, CT], F32, kind="ExternalInput").ap()
    d["gmat"] = nc.dram_tensor("gmat", [128, 128], F32, kind="ExternalInput").ap()
    d["out"] = nc.dram_tensor("out", [C, SH], F32, kind="ExternalOutput").ap()

    with tile.TileContext(nc) as tc:
        with ExitStack() as ctx:
            _build_kernel(ctx, tc, d)
    nc.compile()
    _CACHE["nc"] = nc
    return nc


def make_in_maps(**inputs):
    """Per-core input dicts (numpy). Core c handles batch c//2, query-half c%2."""
    f32 = np.float32
    hs = np.asarray(inputs["hidden_states"], f32).reshape(B, C, S)
    common = {}
    for wname, key, ws in (
        ("wqt", "wq", QKSCALE),
        ("wkt", "wk", QKSCALE),
        ("wvt", "wv", VSCALE),
        ("wot", "wo", OSCALE),
    ):
        w = np.asarray(inputs[key], f32) * ws
        common[wname] = np.ascontiguousarray(w.T).astype(F8NP)
    # bv folds into the output-projection bias: out += (P @ 1*bv) @ Wo.T
    bo_eff = np.asarray(inputs["bo"], f32) + np.asarray(inputs["wo"], f32) @ np.asarray(
        inputs["bv"], f32
    )
    for bname, bvec, bs in (
        ("bq2", np.asarray(inputs["bq"], f32), QKSCALE),
        ("bk2", np.asarray(inputs["bk"], f32), QKSCALE),
        ("bo2", bo_eff, 1.0),
    ):
        b = bvec * bs
        common[bname] = np.ascontiguousarray(b.reshape(CT, 128).T)
    common["gw2"] = np.ascontiguousarray(
        np.asarray(inputs["gn_weight"], f32).reshape(CT, 128).T
    )
    common["gb2"] = np.ascontiguousarray(
        np.asarray(inputs["gn_bias"], f32).reshape(CT, 128).T
    )
    gmat = np.zeros((128, 128), f32)
    for g in range(128 // GSIZE):
        # averages raw per-partition [sum, sumsq] into per-group [mean, E[x^2]]
        gmat[g * GSIZE : (g + 1) * GSIZE, g * GSIZE : (g + 1) * GSIZE] = 1.0 / (
            GSIZE * S
        )
    common["gmat"] = gmat

    in_maps = []
    for core in range(N_CORES):
        b_idx, half = divmod(core, 2)
        xb = hs[b_idx]
        if half:
            xp = np.concatenate([xb[:, SH:], xb[:, :SH]], axis=1)
        else:
            xp = xb
        m = dict(common)
        m["x"] = np.ascontiguousarray(xp.astype(np.float16))
        m["xr"] = np.ascontiguousarray(xp[:, :SH])
        in_maps.append(m)
    return in_maps


def assemble_output(results):
    out = np.empty((B, C, S), np.float32)
    for core in range(N_CORES):
        b_idx, half = divmod(core, 2)
        out[b_idx][:, half * SH : (half + 1) * SH] = results[core]["out"]
    return out.reshape(B, C, 64, 64)


def run(trace=False, **inputs):
    nc = _get_program()
    in_maps = make_in_maps(**inputs)
    res = run_bass_kernel_spmd(nc, in_maps, core_ids=list(range(N_CORES)), trace=trace)
    return assemble_output(res.results), res


def kernel(**inputs):
    out, _ = run(**inputs)
    return out
